# revision 9
# baseline (speedup 1.0000x reference)
"""Attention kernel for trn2: B=4, N=2048, DIM=512, HEADS=8, DIM_HEAD=64.

Sharding: head-parallel across 8 cores (core h computes head h for all 4
batches). Each core returns a partial [4, 2048, 512] bf16 output (its head's
contribution through W_out); the host sums the 8 partials in fp32.

Per-core pipeline (all matmuls bf16, fp32 PSUM accumulate):
  phase 1 (projections, W-stationary so q/k emerge pre-transposed):
    QKc^T = [Wq|Wk]^T x^T and QKs^T = [Wq P|Wk P]^T x^T  (P = rotate-half
    permutation folded into the weights on host), then rotary is just
    rot = QKc*cos + QKs*sin on DVE (position runs along the free axis).
    v is projected x-stationary into natural [n, d] layout. DMA sbuf->sbuf
    remaps build qdup (q^T duplicated into both partition halves) and kTp
    (k^T chunks packed by parity into halves).
  phase 2 (attention, per (batch, 512-wide q tile)):
    S^T pairs via two concurrent K=64 row-tiled matmuls -> 2 psum banks;
    ACT exp over the [128,1024] pair (psum->sbuf bf16); attn = et * expB
    (host-precomputed exp(bias^T) bf16, loaded once per q-tile and shared
    by all 4 batches) on DVE/GpSimd; PV accumulates out^T (+ ones column
    for the softmax denominator); denominator is transposed via K=1
    matmuls to get per-partition reciprocals; W_out projection (K=64) with
    normalization folded into the psum evacuation as a tensor_scalar mul.
"""

import numpy as np

B, N, DIM = 4, 2048, 512
HEADS, DH = 8, 64
P = 128
DC = DIM // P          # 4 dim chunks
NCH = N // P           # 16 n chunks
QT = 512               # q tile in phase 2
NQT = N // QT          # 4
PAIRS = NCH // 2       # 8 k-chunk pairs
NB = N // QT           # 4 n blocks in phase 1
GPS_PAIRS = (2, 3)     # pairs whose bias-multiply runs on GpSimd

_CACHE = {}


def _build():
    import concourse.mybir as mybir
    import concourse.tile as tile
    from concourse import bacc

    F32 = mybir.dt.float32
    BF16 = mybir.dt.bfloat16
    EXP = mybir.ActivationFunctionType.Exp

    nc = bacc.Bacc(None, target_bir_lowering=False)

    # ---- inputs ----
    xT4_d = nc.dram_tensor("xT4", [B, P, DC, N], BF16, kind="ExternalInput")
    wqk_d = nc.dram_tensor("wqk", [P, 2, DC, P], BF16, kind="ExternalInput")
    wv_d = nc.dram_tensor("wv", [P, DC, DH], BF16, kind="ExternalInput")
    wout_d = nc.dram_tensor("wout", [DH, DIM], BF16, kind="ExternalInput")
    expb_d = nc.dram_tensor(
        "expb", [NQT, P, PAIRS, 2, QT], BF16, kind="ExternalInput"
    )
    cos2_d = nc.dram_tensor("cos2", [P, N], BF16, kind="ExternalInput")
    sin2_d = nc.dram_tensor("sin2", [P, N], BF16, kind="ExternalInput")
    onesv_d = nc.dram_tensor("onesv", [P, NCH], BF16, kind="ExternalInput")
    out_d = nc.dram_tensor("out", [B, N, DIM], BF16, kind="ExternalOutput")

    with tile.TileContext(nc) as tc:
        with tc.tile_pool(name="const", bufs=1) as cp:
            wqk_t = cp.tile([P, 2, DC, P], BF16, tag="wqk")
            nc.sync.dma_start(wqk_t[:], wqk_d[:, :, :, :])
            wv_t = cp.tile([P, DC, DH], BF16, tag="wv")
            nc.sync.dma_start(wv_t[:], wv_d[:, :, :])
            wout_t = cp.tile([DH, DIM], BF16, tag="wout")
            nc.sync.dma_start(wout_t[:], wout_d[:, :])
            cos2_t = cp.tile([P, N], BF16, tag="cos2")
            nc.sync.dma_start(cos2_t[:], cos2_d[:, :])
            sin2_t = cp.tile([P, N], BF16, tag="sin2")
            nc.sync.dma_start(sin2_t[:], sin2_d[:, :])
            ones_t = cp.tile([P, NCH], BF16, tag="ones")
            nc.sync.dma_start(ones_t[:], onesv_d[:, :])

            # persistent per-batch activations
            qdup_b = [cp.tile([P, N], BF16, tag=f"qdup{b}", name=f"qdup{b}") for b in range(B)]
            kTp_b = [cp.tile([P, PAIRS, P], BF16, tag=f"kTp{b}", name=f"kTp{b}") for b in range(B)]
            v_b = [cp.tile([P, NCH, DH + 1], BF16, tag=f"v{b}", name=f"v{b}") for b in range(B)]
            for b in range(B):
                nc.sync.dma_start(v_b[b][:, :, DH : DH + 1], onesv_d[:, :, None])

            # ---- fused phase 1 + phase 2 ----
            # phase1(b) is emitted, then phase2(jq=0, b) immediately after, so
            # the scheduler hides projections for batches 1-3 under the
            # ACT-bound attention pipeline of earlier batches.
            with (
                tc.tile_pool(name="p1", bufs=3) as p1,
                tc.tile_pool(name="eb", bufs=2) as ebp,
                tc.tile_pool(name="p2", bufs=3) as p2,
                tc.tile_pool(name="ps_s", bufs=2, space="PSUM") as ps_s,
                tc.tile_pool(name="ps_v", bufs=1, space="PSUM") as ps_v,
                tc.tile_pool(name="ps_o", bufs=2, space="PSUM") as ps_o,
                tc.tile_pool(name="ps_w", bufs=1, space="PSUM") as ps_w,
            ):

                def phase1(b):
                    rot = p1.tile([P, N], BF16, tag="rot", name=f"rot{b}")
                    for nb in range(NB):
                        ns = slice(nb * QT, (nb + 1) * QT)
                        xblk = p1.tile([P, DC, QT], BF16, tag="xblk", name="xblk")
                        nc.sync.dma_start(xblk[:], xT4_d[b, :, :, ns])
                        qk_ps = ps_s.tile([P, 2, QT], F32, tag="s", name="qk_ps")
                        for g in range(2):
                            for dc in range(DC):
                                nc.tensor.matmul(
                                    qk_ps[:, g],
                                    lhsT=wqk_t[:, g, dc],
                                    rhs=xblk[:, dc],
                                    start=(dc == 0),
                                    stop=(dc == DC - 1),
                                )
                        vblk_ps = ps_v.tile([P, 4, DH], F32, tag="vblk", name="vblk")
                        for ci in range(4):
                            for dc in range(DC):
                                nc.tensor.matmul(
                                    vblk_ps[:, ci],
                                    lhsT=xblk[:, dc, ci * P : (ci + 1) * P],
                                    rhs=wv_t[:, dc],
                                    start=(dc == 0),
                                    stop=(dc == DC - 1),
                                )
                        qk_sb = p1.tile([P, 2, QT], BF16, tag="qk_sb", name="qk_sb")
                        nc.vector.tensor_copy(qk_sb[:], qk_ps[:])
                        nc.vector.tensor_copy(
                            v_b[b][:, nb * 4 : nb * 4 + 4, 0:DH], vblk_ps[:]
                        )
                        # rotary: rot = qkc*cos + qks*sin  (bf16, 2x mode)
                        m1 = p1.tile([P, QT], BF16, tag="m1", name="m1")
                        nc.vector.tensor_mul(m1[:], qk_sb[:, 0], cos2_t[:, ns])
                        m2 = p1.tile([P, QT], BF16, tag="m2", name="m2")
                        nc.vector.tensor_mul(m2[:], qk_sb[:, 1], sin2_t[:, ns])
                        nc.vector.tensor_add(rot[:, ns], m1[:], m2[:])
                    # layout remaps via DMA (cross-partition moves)
                    nc.sync.dma_start(qdup_b[b][0:DH, :], rot[0:DH, :])
                    nc.sync.dma_start(qdup_b[b][DH:P, :], rot[0:DH, :])
                    r3 = rot.rearrange("p (pr two f) -> p pr two f", two=2, f=P)
                    nc.sync.dma_start(kTp_b[b][0:DH, :, :], r3[DH:P, :, 0, :])
                    nc.sync.dma_start(kTp_b[b][DH:P, :, :], r3[DH:P, :, 1, :])

                def phase2(jq, b, eb_t):
                    qs = slice(jq * QT, (jq + 1) * QT)
                    outT_ps = ps_o.tile([DH + 1, QT], F32, tag="outT", name="outT")
                    gps_lo = GPS_PAIRS[0] if GPS_PAIRS else -2
                    deferred = []
                    n_pv = 0
                    total_pv = 2 * PAIRS
                    et2 = attn2 = None
                    for pr in range(PAIRS):
                        s_ps = ps_s.tile([P, 2, QT], F32, tag="s", name="s_ps")
                        nc.tensor.matmul(
                            s_ps[:, 0],
                            lhsT=kTp_b[b][0:DH, pr],
                            rhs=qdup_b[b][0:DH, qs],
                            start=True,
                            stop=True,
                            tile_position=(0, 0),
                        )
                        nc.tensor.matmul(
                            s_ps[:, 1],
                            lhsT=kTp_b[b][DH:P, pr],
                            rhs=qdup_b[b][DH:P, qs],
                            start=True,
                            stop=True,
                            tile_position=(64, 0),
                        )
                        if pr in GPS_PAIRS:
                            if pr == gps_lo:
                                et2 = p2.tile([P, 4, QT], BF16, tag="et2", name="et2")
                            half = 2 * (pr - gps_lo)
                            nc.scalar.activation(et2[:, half : half + 2], s_ps[:], EXP)
                            if pr == gps_lo + 1:
                                attn2 = p2.tile(
                                    [P, 4, QT], BF16, tag="attn2", name="attn2"
                                )
                                nc.gpsimd.tensor_mul(
                                    attn2[:], et2[:], eb_t[:, gps_lo : gps_lo + 2]
                                )
                                deferred = [
                                    (2 * gps_lo + j, attn2[:, j]) for j in range(4)
                                ]
                        else:
                            et = p2.tile([P, 2, QT], BF16, tag="et", name="et")
                            nc.scalar.activation(et[:], s_ps[:], EXP)
                            attn = p2.tile([P, 2, QT], BF16, tag="attn", name="attn")
                            nc.vector.tensor_mul(attn[:], et[:], eb_t[:, pr])
                            for j in range(2):
                                n_pv += 1
                                nc.tensor.matmul(
                                    outT_ps[:],
                                    lhsT=v_b[b][:, 2 * pr + j],
                                    rhs=attn[:, j],
                                    start=(n_pv == 1),
                                    stop=(n_pv == total_pv),
                                )
                    for ch, rhs_ap in deferred:
                        n_pv += 1
                        nc.tensor.matmul(
                            outT_ps[:],
                            lhsT=v_b[b][:, ch],
                            rhs=rhs_ap,
                            start=(n_pv == 1),
                            stop=(n_pv == total_pv),
                        )
                    # denominator -> per-partition reciprocal
                    drow = p2.tile([DH + 1, QT], BF16, tag="drow", name="drow")
                    nc.vector.tensor_copy(
                        drow[DH : DH + 1, :], outT_ps[DH : DH + 1, :]
                    )
                    dT_ps = ps_w.tile([P, DIM], F32, tag="wo", name="dTw")
                    for s4 in range(4):
                        nc.tensor.matmul(
                            dT_ps[:, s4 : s4 + 1],
                            lhsT=drow[DH : DH + 1, s4 * P : (s4 + 1) * P],
                            rhs=ones_t[DH : DH + 1, 0:1],
                            start=True,
                            stop=True,
                        )
                    rs = p2.tile([P, 4], F32, tag="rs", name="rs")
                    with nc.allow_low_precision(reason="softmax recip"):
                        nc.vector.reciprocal(rs[:], dT_ps[:, 0:4])
                    ho = p2.tile([DH, QT], BF16, tag="ho", name="ho")
                    nc.vector.tensor_copy(ho[:], outT_ps[0:DH, :])
                    for sq in range(4):
                        wo_ps = ps_w.tile([P, DIM], F32, tag="wo", name="wo")
                        nc.tensor.matmul(
                            wo_ps[:],
                            lhsT=ho[:, sq * P : (sq + 1) * P],
                            rhs=wout_t[:],
                            start=True,
                            stop=True,
                        )
                        ob = p2.tile([P, DIM], BF16, tag="ob", name="ob")
                        if sq % 2 == 0:
                            nc.vector.tensor_scalar_mul(
                                ob[:], wo_ps[:], rs[:, sq : sq + 1]
                            )
                        else:
                            nc.scalar.activation(
                                ob[:],
                                wo_ps[:],
                                mybir.ActivationFunctionType.Copy,
                                scale=rs[:, sq : sq + 1],
                            )
                        row0 = jq * QT + sq * P
                        nc.sync.dma_start(out_d[b, row0 : row0 + P, :], ob[:])

                eb0 = ebp.tile([P, PAIRS, 2, QT], BF16, tag="eb", name="eb0")
                nc.sync.dma_start(eb0[:], expb_d[0])
                for b in range(B):
                    phase1(b)
                    phase2(0, b, eb0)
                for jq in range(1, NQT):
                    eb_t = ebp.tile([P, PAIRS, 2, QT], BF16, tag="eb", name="eb")
                    nc.sync.dma_start(eb_t[:], expb_d[jq])
                    for b in range(B):
                        phase2(jq, b, eb_t)

    nc.compile()
    return nc


def _host_inputs(x, pos_bias, W_qkv, W_out):
    """Build the per-core input maps (pure data marshalling)."""
    import ml_dtypes

    bf16 = ml_dtypes.bfloat16

    xT = np.ascontiguousarray(x.transpose(0, 2, 1))          # [B, DIM, N]
    xT4 = np.ascontiguousarray(
        xT.reshape(B, DC, P, N).transpose(0, 2, 1, 3)
    ).astype(bf16)                                           # [B, P, DC, N]

    # split-d permutation: even dims then odd dims
    perm = np.concatenate([np.arange(0, DH, 2), np.arange(1, DH, 2)])
    inv_freq = (1.0 / (10000.0 ** (np.arange(0, DH, 2, dtype=np.float32) / DH)))
    pos = np.arange(N, dtype=np.float32)
    fr = inv_freq[:, None] * pos[None, :]                     # [32, N]
    cos_h = np.cos(fr)
    sin_h = np.sin(fr)
    # rows: q-even, q-odd, k-even, k-odd halves all share the per-pair angle
    cos2 = np.concatenate([cos_h] * 4, axis=0).astype(bf16)   # [128, N]
    sin2 = np.concatenate([sin_h] * 4, axis=0).astype(bf16)

    onesv = np.ones((P, NCH), dtype=np.float32).astype(bf16)

    scale = np.float32(DH ** -0.5)
    in_maps = []
    for h in range(HEADS):
        Wq = (W_qkv[:, h * DH : (h + 1) * DH] * scale)[:, perm]   # split-d
        Wk = W_qkv[:, DIM + h * DH : DIM + (h + 1) * DH][:, perm]
        Wv = W_qkv[:, 2 * DIM + h * DH : 2 * DIM + (h + 1) * DH]
        # rotate-half in split layout: s_e = -c_o, s_o = c_e
        Wq_s = np.concatenate([-Wq[:, 32:64], Wq[:, 0:32]], axis=1)
        Wk_s = np.concatenate([-Wk[:, 32:64], Wk[:, 0:32]], axis=1)
        Wc = np.concatenate([Wq, Wk], axis=1)                 # [512, 128]
        Ws = np.concatenate([Wq_s, Wk_s], axis=1)             # [512, 128]
        wqk = np.ascontiguousarray(
            np.stack(
                [
                    Wc.reshape(DC, P, P).transpose(1, 0, 2),
                    Ws.reshape(DC, P, P).transpose(1, 0, 2),
                ],
                axis=1,
            )
        ).astype(bf16)                                        # [P, 2, DC, P]
        wv = np.ascontiguousarray(
            Wv.reshape(DC, P, DH).transpose(1, 0, 2)
        ).astype(bf16)                                        # [P, DC, DH]
        wout = np.ascontiguousarray(W_out[h * DH : (h + 1) * DH, :]).astype(bf16)
        ebT = np.exp(pos_bias[h].T.astype(np.float32))        # [k, q]
        expb = np.ascontiguousarray(
            ebT.reshape(PAIRS, 2, P, NQT, QT).transpose(3, 2, 0, 1, 4)
        ).astype(bf16)                                        # [NQT, P, PAIRS, 2, QT]
        in_maps.append(
            {
                "xT4": xT4,
                "wqk": wqk,
                "wv": wv,
                "wout": wout,
                "expb": expb,
                "cos2": cos2,
                "sin2": sin2,
                "onesv": onesv,
            }
        )
    return in_maps


def kernel(x, pos_bias, W_qkv, W_out, _trace=False):
    from concourse.bass_utils import run_bass_kernel_spmd

    x = np.asarray(x, dtype=np.float32)
    pos_bias = np.asarray(pos_bias, dtype=np.float32)
    W_qkv = np.asarray(W_qkv, dtype=np.float32)
    W_out = np.asarray(W_out, dtype=np.float32)

    if "nc" not in _CACHE:
        _CACHE["nc"] = _build()
    nc = _CACHE["nc"]

    in_maps = _host_inputs(x, pos_bias, W_qkv, W_out)
    try:
        res = run_bass_kernel_spmd(
            nc, in_maps, core_ids=list(range(HEADS)), trace=_trace
        )
    except ModuleNotFoundError:
        res = run_bass_kernel_spmd(
            nc, in_maps, core_ids=list(range(HEADS)), trace=False
        )
    out = np.zeros((B, N, DIM), dtype=np.float32)
    for rmap in res.results:
        out += rmap["out"].astype(np.float32)
    if _trace:
        return out, res
    return out


if __name__ == "__main__":
    rng = np.random.default_rng(0)
    x = rng.standard_normal((B, N, DIM), dtype=np.float32)
    pb = rng.standard_normal((HEADS, N, N), dtype=np.float32)
    wq = rng.standard_normal((DIM, 3 * DIM), dtype=np.float32) * DIM**-0.5
    wo = rng.standard_normal((DIM, DIM), dtype=np.float32) * DIM**-0.5
    o = kernel(x, pb, wq, wo)
    print("kernel ran, out std:", o.std())


# revision 11
# speedup vs baseline: 1.1308x; 1.1308x over previous
"""Attention kernel for trn2: B=4, N=2048, DIM=512, HEADS=8, DIM_HEAD=64.

Sharding: head-parallel across 8 cores (core h computes head h for all 4
batches). Each core returns a partial [4, 2048, 512] bf16 output (its head's
contribution through W_out); the host sums the 8 partials in fp32.

Per-core pipeline (all matmuls bf16, fp32 PSUM accumulate):
  phase 1 (projections, W-stationary so q/k emerge pre-transposed):
    QKc^T = [Wq|Wk]^T x^T and QKs^T = [Wq P|Wk P]^T x^T  (P = rotate-half
    permutation folded into the weights on host), then rotary is just
    rot = QKc*cos + QKs*sin on DVE (position runs along the free axis).
    v is projected x-stationary into natural [n, d] layout. DMA sbuf->sbuf
    remaps build qdup (q^T duplicated into both partition halves) and kTp
    (k^T chunks packed by parity into halves).
  phase 2 (attention, per (batch, 512-wide q tile)):
    S^T pairs via two concurrent K=64 row-tiled matmuls -> 2 psum banks;
    ACT exp over the [128,1024] pair (psum->sbuf bf16); attn = et * expB
    (host-precomputed exp(bias^T) bf16, loaded once per q-tile and shared
    by all 4 batches) on DVE/GpSimd; PV accumulates out^T (+ ones column
    for the softmax denominator); denominator is transposed via K=1
    matmuls to get per-partition reciprocals; W_out projection (K=64) with
    normalization folded into the psum evacuation as a tensor_scalar mul.
"""

import numpy as np

B, N, DIM = 4, 2048, 512
HEADS, DH = 8, 64
P = 128
DC = DIM // P          # 4 dim chunks
NCH = N // P           # 16 n chunks
QT = 512               # q tile in phase 2
NQT = N // QT          # 4
PAIRS = NCH // 2       # 8 k-chunk pairs
NB = N // QT           # 4 n blocks in phase 1
GPS_PAIRS = (2, 3)     # pairs whose bias-multiply runs on GpSimd

_CACHE = {}


def _build():
    import concourse.mybir as mybir
    import concourse.tile as tile
    from concourse import bacc

    F32 = mybir.dt.float32
    BF16 = mybir.dt.bfloat16
    EXP = mybir.ActivationFunctionType.Exp

    nc = bacc.Bacc(None, target_bir_lowering=False)

    # ---- inputs ----
    xT4_d = nc.dram_tensor("xT4", [B, P, DC, N], BF16, kind="ExternalInput")
    wqk_d = nc.dram_tensor("wqk", [P, 2, DC, P], BF16, kind="ExternalInput")
    wv_d = nc.dram_tensor("wv", [P, DC, DH], BF16, kind="ExternalInput")
    wout_d = nc.dram_tensor("wout", [DH, DIM], BF16, kind="ExternalInput")
    expb_d = nc.dram_tensor(
        "expb", [NQT, P, PAIRS, 2, QT], BF16, kind="ExternalInput"
    )
    cos2_d = nc.dram_tensor("cos2", [P, N], BF16, kind="ExternalInput")
    sin2_d = nc.dram_tensor("sin2", [P, N], BF16, kind="ExternalInput")
    onesv_d = nc.dram_tensor("onesv", [P, NCH], BF16, kind="ExternalInput")
    out_d = nc.dram_tensor("out", [B, N, DIM], BF16, kind="ExternalOutput")

    with tile.TileContext(nc) as tc:
        with tc.tile_pool(name="const", bufs=1) as cp:
            wqk_t = cp.tile([P, 2, DC, P], BF16, tag="wqk")
            nc.sync.dma_start(wqk_t[:], wqk_d[:, :, :, :])
            wv_t = cp.tile([P, DC, DH], BF16, tag="wv")
            nc.sync.dma_start(wv_t[:], wv_d[:, :, :])
            wout_t = cp.tile([DH, DIM], BF16, tag="wout")
            nc.sync.dma_start(wout_t[:], wout_d[:, :])
            cos2_t = cp.tile([P, N], BF16, tag="cos2")
            nc.sync.dma_start(cos2_t[:], cos2_d[:, :])
            sin2_t = cp.tile([P, N], BF16, tag="sin2")
            nc.sync.dma_start(sin2_t[:], sin2_d[:, :])
            ones_t = cp.tile([P, NCH], BF16, tag="ones")
            nc.sync.dma_start(ones_t[:], onesv_d[:, :])

            # persistent per-batch activations
            qdup_b = [cp.tile([P, N], BF16, tag=f"qdup{b}", name=f"qdup{b}") for b in range(B)]
            kTp_b = [cp.tile([P, PAIRS, P], BF16, tag=f"kTp{b}", name=f"kTp{b}") for b in range(B)]
            v_b = [cp.tile([P, NCH, DH + 1], BF16, tag=f"v{b}", name=f"v{b}") for b in range(B)]
            for b in range(B):
                nc.sync.dma_start(v_b[b][:, :, DH : DH + 1], onesv_d[:, :, None])

            # ---- fused phase 1 + phase 2 ----
            # phase1(b) is emitted, then phase2(jq=0, b) immediately after, so
            # the scheduler hides projections for batches 1-3 under the
            # ACT-bound attention pipeline of earlier batches.
            with (
                tc.tile_pool(name="p1", bufs=3) as p1,
                tc.tile_pool(name="eb", bufs=2) as ebp,
                tc.tile_pool(name="p2", bufs=3) as p2,
                tc.tile_pool(name="psA", bufs=2, space="PSUM") as psA,
                tc.tile_pool(name="psB", bufs=2, space="PSUM") as psB,
            ):

                def phase1(b):
                    rot = p1.tile([P, N], BF16, tag="rot", name=f"rot{b}")
                    for nb in range(NB):
                        ns = slice(nb * QT, (nb + 1) * QT)
                        xblk = p1.tile([P, DC, QT], BF16, tag="xblk", name="xblk")
                        nc.sync.dma_start(xblk[:], xT4_d[b, :, :, ns])
                        qk_ps = psA.tile([P, 2, QT], F32, tag="s", name="qk_ps")
                        for g in range(2):
                            for dc in range(DC):
                                nc.tensor.matmul(
                                    qk_ps[:, g],
                                    lhsT=wqk_t[:, g, dc],
                                    rhs=xblk[:, dc],
                                    start=(dc == 0),
                                    stop=(dc == DC - 1),
                                )
                        vtile = psB.tile([P, DIM], F32, tag="wo", name="vblk")
                        vblk_ps = vtile[:, 0 : 4 * DH].rearrange(
                            "p (a b) -> p a b", a=4
                        )
                        for ci in range(4):
                            for dc in range(DC):
                                nc.tensor.matmul(
                                    vblk_ps[:, ci],
                                    lhsT=xblk[:, dc, ci * P : (ci + 1) * P],
                                    rhs=wv_t[:, dc],
                                    start=(dc == 0),
                                    stop=(dc == DC - 1),
                                )
                        qk_sb = p1.tile([P, 2, QT], BF16, tag="qk_sb", name="qk_sb")
                        nc.vector.tensor_copy(qk_sb[:], qk_ps[:])
                        nc.vector.tensor_copy(
                            v_b[b][:, nb * 4 : nb * 4 + 4, 0:DH], vblk_ps[:]
                        )
                        # rotary: rot = qkc*cos + qks*sin  (bf16, 2x mode)
                        m1 = p1.tile([P, QT], BF16, tag="m1", name="m1")
                        nc.vector.tensor_mul(m1[:], qk_sb[:, 0], cos2_t[:, ns])
                        m2 = p1.tile([P, QT], BF16, tag="m2", name="m2")
                        nc.vector.tensor_mul(m2[:], qk_sb[:, 1], sin2_t[:, ns])
                        nc.vector.tensor_add(rot[:, ns], m1[:], m2[:])
                    # layout remaps via DMA (cross-partition moves)
                    nc.sync.dma_start(qdup_b[b][0:DH, :], rot[0:DH, :])
                    nc.sync.dma_start(qdup_b[b][DH:P, :], rot[0:DH, :])
                    r3 = rot.rearrange("p (pr two f) -> p pr two f", two=2, f=P)
                    nc.sync.dma_start(kTp_b[b][0:DH, :, :], r3[DH:P, :, 0, :])
                    nc.sync.dma_start(kTp_b[b][DH:P, :, :], r3[DH:P, :, 1, :])

                def phase2(jq, b, eb_t):
                    qs = slice(jq * QT, (jq + 1) * QT)
                    outT_ps = psB.tile([DH + 1, QT], F32, tag="outT", name="outT")
                    gps_lo = GPS_PAIRS[0] if GPS_PAIRS else -2
                    deferred = []
                    n_pv = 0
                    total_pv = 2 * PAIRS
                    et2 = attn2 = None
                    for pr in range(PAIRS):
                        s_ps = psA.tile([P, 2, QT], F32, tag="s", name="s_ps")
                        nc.tensor.matmul(
                            s_ps[:, 0],
                            lhsT=kTp_b[b][0:DH, pr],
                            rhs=qdup_b[b][0:DH, qs],
                            start=True,
                            stop=True,
                            tile_position=(0, 0),
                        )
                        nc.tensor.matmul(
                            s_ps[:, 1],
                            lhsT=kTp_b[b][DH:P, pr],
                            rhs=qdup_b[b][DH:P, qs],
                            start=True,
                            stop=True,
                            tile_position=(64, 0),
                        )
                        if pr in GPS_PAIRS:
                            if pr == gps_lo:
                                et2 = p2.tile([P, 4, QT], BF16, tag="et2", name="et2")
                            half = 2 * (pr - gps_lo)
                            nc.scalar.activation(et2[:, half : half + 2], s_ps[:], EXP)
                            if pr == gps_lo + 1:
                                attn2 = p2.tile(
                                    [P, 4, QT], BF16, tag="attn2", name="attn2"
                                )
                                nc.gpsimd.tensor_mul(
                                    attn2[:], et2[:], eb_t[:, gps_lo : gps_lo + 2]
                                )
                                deferred = [
                                    (2 * gps_lo + j, attn2[:, j]) for j in range(4)
                                ]
                        else:
                            et = p2.tile([P, 2, QT], BF16, tag="et", name="et")
                            nc.scalar.activation(et[:], s_ps[:], EXP)
                            attn = p2.tile([P, 2, QT], BF16, tag="attn", name="attn")
                            nc.vector.tensor_mul(attn[:], et[:], eb_t[:, pr])
                            for j in range(2):
                                n_pv += 1
                                nc.tensor.matmul(
                                    outT_ps[:],
                                    lhsT=v_b[b][:, 2 * pr + j],
                                    rhs=attn[:, j],
                                    start=(n_pv == 1),
                                    stop=(n_pv == total_pv),
                                )
                    for ch, rhs_ap in deferred:
                        n_pv += 1
                        nc.tensor.matmul(
                            outT_ps[:],
                            lhsT=v_b[b][:, ch],
                            rhs=rhs_ap,
                            start=(n_pv == 1),
                            stop=(n_pv == total_pv),
                        )
                    # denominator -> per-partition reciprocal
                    drow = p2.tile([DH + 1, QT], BF16, tag="drow", name="drow")
                    nc.vector.tensor_copy(
                        drow[DH : DH + 1, :], outT_ps[DH : DH + 1, :]
                    )
                    dT_ps = psB.tile([P, DIM], F32, tag="wo", name="dTw")
                    for s4 in range(4):
                        nc.tensor.matmul(
                            dT_ps[:, s4 : s4 + 1],
                            lhsT=drow[DH : DH + 1, s4 * P : (s4 + 1) * P],
                            rhs=ones_t[DH : DH + 1, 0:1],
                            start=True,
                            stop=True,
                        )
                    rs = p2.tile([P, 4], F32, tag="rs", name="rs")
                    with nc.allow_low_precision(reason="softmax recip"):
                        nc.vector.reciprocal(rs[:], dT_ps[:, 0:4])
                    ho = p2.tile([DH, QT], BF16, tag="ho", name="ho")
                    nc.vector.tensor_copy(ho[:], outT_ps[0:DH, :])
                    for sq in range(4):
                        wo_ps = psB.tile([P, DIM], F32, tag="wo", name="wo")
                        nc.tensor.matmul(
                            wo_ps[:],
                            lhsT=ho[:, sq * P : (sq + 1) * P],
                            rhs=wout_t[:],
                            start=True,
                            stop=True,
                        )
                        ob = p2.tile([P, DIM], BF16, tag="ob", name="ob")
                        if sq % 2 == 0:
                            nc.vector.tensor_scalar_mul(
                                ob[:], wo_ps[:], rs[:, sq : sq + 1]
                            )
                        else:
                            nc.scalar.activation(
                                ob[:],
                                wo_ps[:],
                                mybir.ActivationFunctionType.Copy,
                                scale=rs[:, sq : sq + 1],
                            )
                        row0 = jq * QT + sq * P
                        nc.sync.dma_start(out_d[b, row0 : row0 + P, :], ob[:])

                for b in range(B):
                    phase1(b)
                for jq in range(NQT):
                    eb_t = ebp.tile([P, PAIRS, 2, QT], BF16, tag="eb", name="eb")
                    nc.sync.dma_start(eb_t[:], expb_d[jq])
                    for b in range(B):
                        phase2(jq, b, eb_t)

    nc.compile()
    return nc


def _host_inputs(x, pos_bias, W_qkv, W_out):
    """Build the per-core input maps (pure data marshalling)."""
    import ml_dtypes

    bf16 = ml_dtypes.bfloat16

    xT = np.ascontiguousarray(x.transpose(0, 2, 1))          # [B, DIM, N]
    xT4 = np.ascontiguousarray(
        xT.reshape(B, DC, P, N).transpose(0, 2, 1, 3)
    ).astype(bf16)                                           # [B, P, DC, N]

    # split-d permutation: even dims then odd dims
    perm = np.concatenate([np.arange(0, DH, 2), np.arange(1, DH, 2)])
    inv_freq = (1.0 / (10000.0 ** (np.arange(0, DH, 2, dtype=np.float32) / DH)))
    pos = np.arange(N, dtype=np.float32)
    fr = inv_freq[:, None] * pos[None, :]                     # [32, N]
    cos_h = np.cos(fr)
    sin_h = np.sin(fr)
    # rows: q-even, q-odd, k-even, k-odd halves all share the per-pair angle
    cos2 = np.concatenate([cos_h] * 4, axis=0).astype(bf16)   # [128, N]
    sin2 = np.concatenate([sin_h] * 4, axis=0).astype(bf16)

    onesv = np.ones((P, NCH), dtype=np.float32).astype(bf16)

    scale = np.float32(DH ** -0.5)
    in_maps = []
    for h in range(HEADS):
        Wq = (W_qkv[:, h * DH : (h + 1) * DH] * scale)[:, perm]   # split-d
        Wk = W_qkv[:, DIM + h * DH : DIM + (h + 1) * DH][:, perm]
        Wv = W_qkv[:, 2 * DIM + h * DH : 2 * DIM + (h + 1) * DH]
        # rotate-half in split layout: s_e = -c_o, s_o = c_e
        Wq_s = np.concatenate([-Wq[:, 32:64], Wq[:, 0:32]], axis=1)
        Wk_s = np.concatenate([-Wk[:, 32:64], Wk[:, 0:32]], axis=1)
        Wc = np.concatenate([Wq, Wk], axis=1)                 # [512, 128]
        Ws = np.concatenate([Wq_s, Wk_s], axis=1)             # [512, 128]
        wqk = np.ascontiguousarray(
            np.stack(
                [
                    Wc.reshape(DC, P, P).transpose(1, 0, 2),
                    Ws.reshape(DC, P, P).transpose(1, 0, 2),
                ],
                axis=1,
            )
        ).astype(bf16)                                        # [P, 2, DC, P]
        wv = np.ascontiguousarray(
            Wv.reshape(DC, P, DH).transpose(1, 0, 2)
        ).astype(bf16)                                        # [P, DC, DH]
        wout = np.ascontiguousarray(W_out[h * DH : (h + 1) * DH, :]).astype(bf16)
        ebT = np.exp(pos_bias[h].T.astype(np.float32))        # [k, q]
        expb = np.ascontiguousarray(
            ebT.reshape(PAIRS, 2, P, NQT, QT).transpose(3, 2, 0, 1, 4)
        ).astype(bf16)                                        # [NQT, P, PAIRS, 2, QT]
        in_maps.append(
            {
                "xT4": xT4,
                "wqk": wqk,
                "wv": wv,
                "wout": wout,
                "expb": expb,
                "cos2": cos2,
                "sin2": sin2,
                "onesv": onesv,
            }
        )
    return in_maps


def kernel(x, pos_bias, W_qkv, W_out, _trace=False):
    from concourse.bass_utils import run_bass_kernel_spmd

    x = np.asarray(x, dtype=np.float32)
    pos_bias = np.asarray(pos_bias, dtype=np.float32)
    W_qkv = np.asarray(W_qkv, dtype=np.float32)
    W_out = np.asarray(W_out, dtype=np.float32)

    if "nc" not in _CACHE:
        _CACHE["nc"] = _build()
    nc = _CACHE["nc"]

    in_maps = _host_inputs(x, pos_bias, W_qkv, W_out)
    try:
        res = run_bass_kernel_spmd(
            nc, in_maps, core_ids=list(range(HEADS)), trace=_trace
        )
    except ModuleNotFoundError:
        res = run_bass_kernel_spmd(
            nc, in_maps, core_ids=list(range(HEADS)), trace=False
        )
    out = np.zeros((B, N, DIM), dtype=np.float32)
    for rmap in res.results:
        out += rmap["out"].astype(np.float32)
    if _trace:
        return out, res
    return out


if __name__ == "__main__":
    rng = np.random.default_rng(0)
    x = rng.standard_normal((B, N, DIM), dtype=np.float32)
    pb = rng.standard_normal((HEADS, N, N), dtype=np.float32)
    wq = rng.standard_normal((DIM, 3 * DIM), dtype=np.float32) * DIM**-0.5
    wo = rng.standard_normal((DIM, DIM), dtype=np.float32) * DIM**-0.5
    o = kernel(x, pb, wq, wo)
    print("kernel ran, out std:", o.std())


# revision 15
# speedup vs baseline: 1.3822x; 1.2222x over previous
"""Attention kernel for trn2: B=4, N=2048, DIM=512, HEADS=8, DIM_HEAD=64.

Sharding: head-parallel across 8 cores (core h computes head h for all 4
batches). Each core returns a partial [4, 2048, 512] bf16 output (its head's
contribution through W_out); the host sums the 8 partials in fp32.

Per-core pipeline (all matmuls bf16, fp32 PSUM accumulate):
  phase 1 (projections, W-stationary so q/k emerge pre-transposed):
    QKc^T = [Wq|Wk]^T x^T and QKs^T = [Wq P|Wk P]^T x^T  (P = rotate-half
    permutation folded into the weights on host), then rotary is just
    rot = QKc*cos + QKs*sin on DVE (position runs along the free axis).
    v is projected x-stationary into natural [n, d] layout. DMA sbuf->sbuf
    remaps build qdup (q^T duplicated into both partition halves) and kTp
    (k^T chunks packed by parity into halves).
  phase 2 (attention, per (batch, 512-wide q tile)):
    S^T pairs via two concurrent K=64 row-tiled matmuls -> 2 psum banks;
    ACT exp over the [128,1024] pair (psum->sbuf bf16); attn = et * expB
    (host-precomputed exp(bias^T) bf16, loaded once per q-tile and shared
    by all 4 batches) on DVE/GpSimd; PV accumulates out^T (+ ones column
    for the softmax denominator); denominator is transposed via K=1
    matmuls to get per-partition reciprocals; W_out projection (K=64) with
    normalization folded into the psum evacuation as a tensor_scalar mul.
"""

import numpy as np

B, N, DIM = 4, 2048, 512
HEADS, DH = 8, 64
P = 128
DC = DIM // P          # 4 dim chunks
NCH = N // P           # 16 n chunks
QT = 512               # q tile in phase 2
NQT = N // QT          # 4
PAIRS = NCH // 2       # 8 k-chunk pairs
NB = N // QT           # 4 n blocks in phase 1
GPS_PAIRS = ()     # pairs whose bias-multiply runs on GpSimd

_CACHE = {}


def _build():
    import concourse.mybir as mybir
    import concourse.tile as tile
    from concourse import bacc

    F32 = mybir.dt.float32
    BF16 = mybir.dt.bfloat16
    EXP = mybir.ActivationFunctionType.Exp

    nc = bacc.Bacc(None, target_bir_lowering=False)

    # ---- inputs ----
    xT4_d = nc.dram_tensor("xT4", [B, P, DC, N], BF16, kind="ExternalInput")
    wqk_d = nc.dram_tensor("wqk", [P, 2, DC, P], BF16, kind="ExternalInput")
    wv_d = nc.dram_tensor("wv", [P, DC, DH], BF16, kind="ExternalInput")
    wout_d = nc.dram_tensor("wout", [DH, DIM], BF16, kind="ExternalInput")
    expb_d = nc.dram_tensor(
        "expb", [NQT, P, PAIRS, 2, QT], BF16, kind="ExternalInput"
    )
    cos2_d = nc.dram_tensor("cos2", [P, N], BF16, kind="ExternalInput")
    sin2_d = nc.dram_tensor("sin2", [P, N], BF16, kind="ExternalInput")
    onesv_d = nc.dram_tensor("onesv", [P, NCH], BF16, kind="ExternalInput")
    out_d = nc.dram_tensor("out", [B, N, DIM], BF16, kind="ExternalOutput")

    with tile.TileContext(nc) as tc:
        with tc.tile_pool(name="const", bufs=1) as cp:
            wqk_t = cp.tile([P, 2, DC, P], BF16, tag="wqk")
            nc.sync.dma_start(wqk_t[:], wqk_d[:, :, :, :])
            wv_t = cp.tile([P, DC, DH], BF16, tag="wv")
            nc.sync.dma_start(wv_t[:], wv_d[:, :, :])
            wout_t = cp.tile([DH, DIM], BF16, tag="wout")
            nc.sync.dma_start(wout_t[:], wout_d[:, :])
            cos2_t = cp.tile([P, N], BF16, tag="cos2")
            nc.sync.dma_start(cos2_t[:], cos2_d[:, :])
            sin2_t = cp.tile([P, N], BF16, tag="sin2")
            nc.sync.dma_start(sin2_t[:], sin2_d[:, :])
            ones_t = cp.tile([P, NCH], BF16, tag="ones")
            nc.sync.dma_start(ones_t[:], onesv_d[:, :])

            # persistent per-batch activations
            qdup_b = [cp.tile([P, N], BF16, tag=f"qdup{b}", name=f"qdup{b}") for b in range(B)]
            kTp_b = [cp.tile([P, PAIRS, P], BF16, tag=f"kTp{b}", name=f"kTp{b}") for b in range(B)]
            v_b = [cp.tile([P, NCH, DH + 1], BF16, tag=f"v{b}", name=f"v{b}") for b in range(B)]
            for b in range(B):
                nc.sync.dma_start(v_b[b][:, :, DH : DH + 1], onesv_d[:, :, None])

            # ---- fused phase 1 + phase 2 ----
            # phase1(b) is emitted, then phase2(jq=0, b) immediately after, so
            # the scheduler hides projections for batches 1-3 under the
            # ACT-bound attention pipeline of earlier batches.
            with (
                tc.tile_pool(name="p1", bufs=3) as p1,
                tc.tile_pool(name="eb", bufs=2) as ebp,
                tc.tile_pool(name="p2", bufs=3) as p2,
                tc.tile_pool(name="psA", bufs=2, space="PSUM") as psA,
                tc.tile_pool(name="psB", bufs=2, space="PSUM") as psB,
            ):

                def phase1(b):
                    rot = p1.tile([P, N], BF16, tag="rot", name=f"rot{b}")
                    for nb in range(NB):
                        ns = slice(nb * QT, (nb + 1) * QT)
                        xblk = p1.tile([P, DC, QT], BF16, tag="xblk", name="xblk")
                        nc.sync.dma_start(xblk[:], xT4_d[b, :, :, ns])
                        qk_ps = psA.tile([P, 2, QT], F32, tag="s", name="qk_ps")
                        for g in range(2):
                            for dc in range(DC):
                                nc.tensor.matmul(
                                    qk_ps[:, g],
                                    lhsT=wqk_t[:, g, dc],
                                    rhs=xblk[:, dc],
                                    start=(dc == 0),
                                    stop=(dc == DC - 1),
                                )
                        vtile = psB.tile([P, DIM], F32, tag="wo", name="vblk")
                        vblk_ps = vtile[:, 0 : 4 * DH].rearrange(
                            "p (a b) -> p a b", a=4
                        )
                        for ci in range(4):
                            for dc in range(DC):
                                nc.tensor.matmul(
                                    vblk_ps[:, ci],
                                    lhsT=xblk[:, dc, ci * P : (ci + 1) * P],
                                    rhs=wv_t[:, dc],
                                    start=(dc == 0),
                                    stop=(dc == DC - 1),
                                )
                        nc.vector.tensor_copy(
                            v_b[b][:, nb * 4 : nb * 4 + 4, 0:DH], vblk_ps[:]
                        )
                        # rotary: rot = qkc*cos + qks*sin (muls read psum direct)
                        m1 = p1.tile([P, QT], BF16, tag="m1", name="m1")
                        nc.vector.tensor_mul(m1[:], qk_ps[:, 0], cos2_t[:, ns])
                        m2 = p1.tile([P, QT], BF16, tag="m2", name="m2")
                        nc.vector.tensor_mul(m2[:], qk_ps[:, 1], sin2_t[:, ns])
                        nc.vector.tensor_add(rot[:, ns], m1[:], m2[:])
                    # layout remaps via DMA (cross-partition, contiguous)
                    nc.sync.dma_start(qdup_b[b][0:DH, :], rot[0:DH, :])
                    nc.sync.dma_start(qdup_b[b][DH:P, :], rot[0:DH, :])
                    nc.sync.dma_start(
                        kTp_b[b][0:DH, :, :], rot[DH:P, 0 : PAIRS * P]
                    )
                    nc.sync.dma_start(
                        kTp_b[b][DH:P, :, :], rot[DH:P, PAIRS * P : N]
                    )

                def phase2(jq, b, eb_t):
                    qs = slice(jq * QT, (jq + 1) * QT)
                    outT_ps = psB.tile([DH + 1, QT], F32, tag="outT", name="outT")
                    gps_lo = GPS_PAIRS[0] if GPS_PAIRS else -2
                    deferred = []
                    n_pv = 0
                    total_pv = 2 * PAIRS
                    et2 = attn2 = None
                    for pr in range(PAIRS):
                        s_ps = psA.tile([P, 2, QT], F32, tag="s", name="s_ps")
                        nc.tensor.matmul(
                            s_ps[:, 0],
                            lhsT=kTp_b[b][0:DH, pr],
                            rhs=qdup_b[b][0:DH, qs],
                            start=True,
                            stop=True,
                            tile_position=(0, 0),
                        )
                        nc.tensor.matmul(
                            s_ps[:, 1],
                            lhsT=kTp_b[b][DH:P, pr],
                            rhs=qdup_b[b][DH:P, qs],
                            start=True,
                            stop=True,
                            tile_position=(64, 0),
                        )
                        if pr in GPS_PAIRS:
                            if pr == gps_lo:
                                et2 = p2.tile([P, 4, QT], BF16, tag="et2", name="et2")
                            half = 2 * (pr - gps_lo)
                            nc.scalar.activation(et2[:, half : half + 2], s_ps[:], EXP)
                            if pr == gps_lo + 1:
                                attn2 = p2.tile(
                                    [P, 4, QT], BF16, tag="attn2", name="attn2"
                                )
                                nc.gpsimd.tensor_mul(
                                    attn2[:], et2[:], eb_t[:, gps_lo : gps_lo + 2]
                                )
                                deferred = [
                                    (gps_lo + PAIRS * (j % 2) + (j // 2), attn2[:, j]) for j in range(4)
                                ]
                        else:
                            et = p2.tile([P, 2, QT], BF16, tag="et", name="et")
                            nc.scalar.activation(et[:], s_ps[:], EXP)
                            attn = p2.tile([P, 2, QT], BF16, tag="attn", name="attn")
                            nc.vector.tensor_mul(attn[:], et[:], eb_t[:, pr])
                            for j in range(2):
                                n_pv += 1
                                nc.tensor.matmul(
                                    outT_ps[:],
                                    lhsT=v_b[b][:, pr + PAIRS * j],
                                    rhs=attn[:, j],
                                    start=(n_pv == 1),
                                    stop=(n_pv == total_pv),
                                )
                    for ch, rhs_ap in deferred:
                        n_pv += 1
                        nc.tensor.matmul(
                            outT_ps[:],
                            lhsT=v_b[b][:, ch],
                            rhs=rhs_ap,
                            start=(n_pv == 1),
                            stop=(n_pv == total_pv),
                        )
                    # evacuate out^T + denominator row in one copy
                    hod = p2.tile([DH + 1, QT], BF16, tag="hod", name="hod")
                    nc.vector.tensor_copy(hod[:], outT_ps[:])
                    dT_ps = psB.tile([P, DIM], F32, tag="wo", name="dTw")
                    for s4 in range(4):
                        nc.tensor.matmul(
                            dT_ps[:, s4 : s4 + 1],
                            lhsT=hod[DH : DH + 1, s4 * P : (s4 + 1) * P],
                            rhs=ones_t[DH : DH + 1, 0:1],
                            start=True,
                            stop=True,
                        )
                    rs = p2.tile([P, 4], F32, tag="rs", name="rs")
                    with nc.allow_low_precision(reason="softmax recip"):
                        nc.vector.reciprocal(rs[:], dT_ps[:, 0:4])
                    ho = hod
                    for sq in range(4):
                        wo_ps = psB.tile([P, DIM], F32, tag="wo", name="wo")
                        nc.tensor.matmul(
                            wo_ps[:],
                            lhsT=ho[0:DH, sq * P : (sq + 1) * P],
                            rhs=wout_t[:],
                            start=True,
                            stop=True,
                        )
                        ob = p2.tile([P, DIM], BF16, tag="ob", name="ob")
                        nc.vector.tensor_scalar_mul(
                            ob[:], wo_ps[:], rs[:, sq : sq + 1]
                        )
                        row0 = jq * QT + sq * P
                        nc.sync.dma_start(out_d[b, row0 : row0 + P, :], ob[:])

                for b in range(B):
                    phase1(b)
                for jq in range(NQT):
                    eb_t = ebp.tile([P, PAIRS, 2, QT], BF16, tag="eb", name="eb")
                    nc.sync.dma_start(eb_t[:], expb_d[jq])
                    for b in range(B):
                        phase2(jq, b, eb_t)

    nc.compile()
    return nc


def _host_inputs(x, pos_bias, W_qkv, W_out):
    """Build the per-core input maps (pure data marshalling)."""
    import ml_dtypes

    bf16 = ml_dtypes.bfloat16

    xT = np.ascontiguousarray(x.transpose(0, 2, 1))          # [B, DIM, N]
    xT4 = np.ascontiguousarray(
        xT.reshape(B, DC, P, N).transpose(0, 2, 1, 3)
    ).astype(bf16)                                           # [B, P, DC, N]

    # split-d permutation: even dims then odd dims
    perm = np.concatenate([np.arange(0, DH, 2), np.arange(1, DH, 2)])
    inv_freq = (1.0 / (10000.0 ** (np.arange(0, DH, 2, dtype=np.float32) / DH)))
    pos = np.arange(N, dtype=np.float32)
    fr = inv_freq[:, None] * pos[None, :]                     # [32, N]
    cos_h = np.cos(fr)
    sin_h = np.sin(fr)
    # rows: q-even, q-odd, k-even, k-odd halves all share the per-pair angle
    cos2 = np.concatenate([cos_h] * 4, axis=0).astype(bf16)   # [128, N]
    sin2 = np.concatenate([sin_h] * 4, axis=0).astype(bf16)

    onesv = np.ones((P, NCH), dtype=np.float32).astype(bf16)

    scale = np.float32(DH ** -0.5)
    in_maps = []
    for h in range(HEADS):
        Wq = (W_qkv[:, h * DH : (h + 1) * DH] * scale)[:, perm]   # split-d
        Wk = W_qkv[:, DIM + h * DH : DIM + (h + 1) * DH][:, perm]
        Wv = W_qkv[:, 2 * DIM + h * DH : 2 * DIM + (h + 1) * DH]
        # rotate-half in split layout: s_e = -c_o, s_o = c_e
        Wq_s = np.concatenate([-Wq[:, 32:64], Wq[:, 0:32]], axis=1)
        Wk_s = np.concatenate([-Wk[:, 32:64], Wk[:, 0:32]], axis=1)
        Wc = np.concatenate([Wq, Wk], axis=1)                 # [512, 128]
        Ws = np.concatenate([Wq_s, Wk_s], axis=1)             # [512, 128]
        wqk = np.ascontiguousarray(
            np.stack(
                [
                    Wc.reshape(DC, P, P).transpose(1, 0, 2),
                    Ws.reshape(DC, P, P).transpose(1, 0, 2),
                ],
                axis=1,
            )
        ).astype(bf16)                                        # [P, 2, DC, P]
        wv = np.ascontiguousarray(
            Wv.reshape(DC, P, DH).transpose(1, 0, 2)
        ).astype(bf16)                                        # [P, DC, DH]
        wout = np.ascontiguousarray(W_out[h * DH : (h + 1) * DH, :]).astype(bf16)
        ebT = np.exp(pos_bias[h].T.astype(np.float32))        # [k, q]
        expb = np.ascontiguousarray(
            ebT.reshape(2, PAIRS, P, NQT, QT).transpose(3, 2, 1, 0, 4)
        ).astype(bf16)                                        # [NQT, P, PAIRS, 2, QT]
        in_maps.append(
            {
                "xT4": xT4,
                "wqk": wqk,
                "wv": wv,
                "wout": wout,
                "expb": expb,
                "cos2": cos2,
                "sin2": sin2,
                "onesv": onesv,
            }
        )
    return in_maps


def kernel(x, pos_bias, W_qkv, W_out, _trace=False):
    from concourse.bass_utils import run_bass_kernel_spmd

    x = np.asarray(x, dtype=np.float32)
    pos_bias = np.asarray(pos_bias, dtype=np.float32)
    W_qkv = np.asarray(W_qkv, dtype=np.float32)
    W_out = np.asarray(W_out, dtype=np.float32)

    if "nc" not in _CACHE:
        _CACHE["nc"] = _build()
    nc = _CACHE["nc"]

    in_maps = _host_inputs(x, pos_bias, W_qkv, W_out)
    try:
        res = run_bass_kernel_spmd(
            nc, in_maps, core_ids=list(range(HEADS)), trace=_trace
        )
    except ModuleNotFoundError:
        res = run_bass_kernel_spmd(
            nc, in_maps, core_ids=list(range(HEADS)), trace=False
        )
    out = np.zeros((B, N, DIM), dtype=np.float32)
    for rmap in res.results:
        out += rmap["out"].astype(np.float32)
    if _trace:
        return out, res
    return out


if __name__ == "__main__":
    rng = np.random.default_rng(0)
    x = rng.standard_normal((B, N, DIM), dtype=np.float32)
    pb = rng.standard_normal((HEADS, N, N), dtype=np.float32)
    wq = rng.standard_normal((DIM, 3 * DIM), dtype=np.float32) * DIM**-0.5
    wo = rng.standard_normal((DIM, DIM), dtype=np.float32) * DIM**-0.5
    o = kernel(x, pb, wq, wo)
    print("kernel ran, out std:", o.std())


# revision 16
# speedup vs baseline: 1.4469x; 1.0468x over previous
"""Attention kernel for trn2: B=4, N=2048, DIM=512, HEADS=8, DIM_HEAD=64.

Sharding: head-parallel across 8 cores (core h computes head h for all 4
batches). Each core returns a partial [4, 2048, 512] bf16 output (its head's
contribution through W_out); the host sums the 8 partials in fp32.

Per-core pipeline (all matmuls bf16, fp32 PSUM accumulate):
  phase 1 (projections, W-stationary so q/k emerge pre-transposed):
    QKc^T = [Wq|Wk]^T x^T and QKs^T = [Wq P|Wk P]^T x^T  (P = rotate-half
    permutation folded into the weights on host), then rotary is just
    rot = QKc*cos + QKs*sin on DVE (position runs along the free axis).
    v is projected x-stationary into natural [n, d] layout. DMA sbuf->sbuf
    remaps build qdup (q^T duplicated into both partition halves) and kTp
    (k^T chunks packed by parity into halves).
  phase 2 (attention, per (batch, 512-wide q tile)):
    S^T pairs via two concurrent K=64 row-tiled matmuls -> 2 psum banks;
    ACT exp over the [128,1024] pair (psum->sbuf bf16); attn = et * expB
    (host-precomputed exp(bias^T) bf16, loaded once per q-tile and shared
    by all 4 batches) on DVE/GpSimd; PV accumulates out^T (+ ones column
    for the softmax denominator); denominator is transposed via K=1
    matmuls to get per-partition reciprocals; W_out projection (K=64) with
    normalization folded into the psum evacuation as a tensor_scalar mul.
"""

import numpy as np

B, N, DIM = 4, 2048, 512
HEADS, DH = 8, 64
P = 128
DC = DIM // P          # 4 dim chunks
NCH = N // P           # 16 n chunks
QT = 512               # q tile in phase 2
NQT = N // QT          # 4
PAIRS = NCH // 2       # 8 k-chunk pairs
NB = N // QT           # 4 n blocks in phase 1
GPS_PAIRS = ()     # pairs whose bias-multiply runs on GpSimd

_CACHE = {}


def _build():
    import concourse.mybir as mybir
    import concourse.tile as tile
    from concourse import bacc

    F32 = mybir.dt.float32
    BF16 = mybir.dt.bfloat16
    EXP = mybir.ActivationFunctionType.Exp

    nc = bacc.Bacc(None, target_bir_lowering=False)

    # ---- inputs ----
    xT4_d = nc.dram_tensor("xT4", [B, P, DC, N], BF16, kind="ExternalInput")
    wqk_d = nc.dram_tensor("wqk", [P, 2, DC, P], BF16, kind="ExternalInput")
    wv_d = nc.dram_tensor("wv", [P, DC, DH], BF16, kind="ExternalInput")
    wout_d = nc.dram_tensor("wout", [DH, DIM], BF16, kind="ExternalInput")
    expb_d = nc.dram_tensor(
        "expb", [NQT, P, PAIRS, 2, QT], BF16, kind="ExternalInput"
    )
    cos2_d = nc.dram_tensor("cos2", [P, N], BF16, kind="ExternalInput")
    sin2_d = nc.dram_tensor("sin2", [P, N], BF16, kind="ExternalInput")
    onesv_d = nc.dram_tensor("onesv", [P, NCH], BF16, kind="ExternalInput")
    out_d = nc.dram_tensor("out", [B, N, DIM], BF16, kind="ExternalOutput")

    with tile.TileContext(nc) as tc:
        with tc.tile_pool(name="const", bufs=1) as cp:
            wqk_t = cp.tile([P, 2, DC, P], BF16, tag="wqk")
            nc.sync.dma_start(wqk_t[:], wqk_d[:, :, :, :])
            wv_t = cp.tile([P, DC, DH], BF16, tag="wv")
            nc.sync.dma_start(wv_t[:], wv_d[:, :, :])
            wout_t = cp.tile([DH, DIM], BF16, tag="wout")
            nc.sync.dma_start(wout_t[:], wout_d[:, :])
            cos2_t = cp.tile([P, N], BF16, tag="cos2")
            nc.sync.dma_start(cos2_t[:], cos2_d[:, :])
            sin2_t = cp.tile([P, N], BF16, tag="sin2")
            nc.sync.dma_start(sin2_t[:], sin2_d[:, :])
            ones_t = cp.tile([P, NCH], BF16, tag="ones")
            nc.sync.dma_start(ones_t[:], onesv_d[:, :])

            # persistent per-batch activations
            qdup_b = [cp.tile([P, N], BF16, tag=f"qdup{b}", name=f"qdup{b}") for b in range(B)]
            kTp_b = [cp.tile([P, PAIRS, P], BF16, tag=f"kTp{b}", name=f"kTp{b}") for b in range(B)]
            v_b = [cp.tile([P, NCH, DH + 1], BF16, tag=f"v{b}", name=f"v{b}") for b in range(B)]
            for b in range(B):
                nc.sync.dma_start(v_b[b][:, :, DH : DH + 1], onesv_d[:, :, None])

            # ---- fused phase 1 + phase 2 ----
            # phase1(b) is emitted, then phase2(jq=0, b) immediately after, so
            # the scheduler hides projections for batches 1-3 under the
            # ACT-bound attention pipeline of earlier batches.
            with (
                tc.tile_pool(name="p1", bufs=3) as p1,
                tc.tile_pool(name="p1x", bufs=5) as p1x,
                tc.tile_pool(name="eb", bufs=2) as ebp,
                tc.tile_pool(name="p2", bufs=3) as p2,
                tc.tile_pool(name="psA", bufs=2, space="PSUM") as psA,
                tc.tile_pool(name="psB", bufs=2, space="PSUM") as psB,
            ):

                def phase1(b):
                    rot = p1.tile([P, N], BF16, tag="rot", name=f"rot{b}")
                    for nb in range(NB):
                        ns = slice(nb * QT, (nb + 1) * QT)
                        xblk = p1x.tile([P, DC, QT], BF16, tag="xblk", name="xblk")
                        nc.sync.dma_start(xblk[:], xT4_d[b, :, :, ns])
                        qk_ps = psA.tile([P, 2, QT], F32, tag="s", name="qk_ps")
                        for g in range(2):
                            for dc in range(DC):
                                nc.tensor.matmul(
                                    qk_ps[:, g],
                                    lhsT=wqk_t[:, g, dc],
                                    rhs=xblk[:, dc],
                                    start=(dc == 0),
                                    stop=(dc == DC - 1),
                                )
                        vtile = psB.tile([P, DIM], F32, tag="wo", name="vblk")
                        vblk_ps = vtile[:, 0 : 4 * DH].rearrange(
                            "p (a b) -> p a b", a=4
                        )
                        for ci in range(4):
                            for dc in range(DC):
                                nc.tensor.matmul(
                                    vblk_ps[:, ci],
                                    lhsT=xblk[:, dc, ci * P : (ci + 1) * P],
                                    rhs=wv_t[:, dc],
                                    start=(dc == 0),
                                    stop=(dc == DC - 1),
                                )
                        nc.vector.tensor_copy(
                            v_b[b][:, nb * 4 : nb * 4 + 4, 0:DH], vblk_ps[:]
                        )
                        # rotary: rot = qkc*cos + qks*sin (muls read psum direct)
                        m1 = p1.tile([P, QT], BF16, tag="m1", name="m1")
                        nc.vector.tensor_mul(m1[:], qk_ps[:, 0], cos2_t[:, ns])
                        m2 = p1.tile([P, QT], BF16, tag="m2", name="m2")
                        nc.vector.tensor_mul(m2[:], qk_ps[:, 1], sin2_t[:, ns])
                        nc.vector.tensor_add(rot[:, ns], m1[:], m2[:])
                    # layout remaps via DMA on the scalar-engine queue so
                    # they don't block the next batch's x loads on sync
                    nc.scalar.dma_start(qdup_b[b][0:DH, :], rot[0:DH, :])
                    nc.scalar.dma_start(qdup_b[b][DH:P, :], rot[0:DH, :])
                    nc.scalar.dma_start(
                        kTp_b[b][0:DH, :, :], rot[DH:P, 0 : PAIRS * P]
                    )
                    nc.scalar.dma_start(
                        kTp_b[b][DH:P, :, :], rot[DH:P, PAIRS * P : N]
                    )

                def phase2(jq, b, eb_t):
                    qs = slice(jq * QT, (jq + 1) * QT)
                    outT_ps = psB.tile([DH + 1, QT], F32, tag="outT", name="outT")
                    gps_lo = GPS_PAIRS[0] if GPS_PAIRS else -2
                    deferred = []
                    n_pv = 0
                    total_pv = 2 * PAIRS
                    et2 = attn2 = None
                    for pr in range(PAIRS):
                        s_ps = psA.tile([P, 2, QT], F32, tag="s", name="s_ps")
                        nc.tensor.matmul(
                            s_ps[:, 0],
                            lhsT=kTp_b[b][0:DH, pr],
                            rhs=qdup_b[b][0:DH, qs],
                            start=True,
                            stop=True,
                            tile_position=(0, 0),
                        )
                        nc.tensor.matmul(
                            s_ps[:, 1],
                            lhsT=kTp_b[b][DH:P, pr],
                            rhs=qdup_b[b][DH:P, qs],
                            start=True,
                            stop=True,
                            tile_position=(64, 0),
                        )
                        if pr in GPS_PAIRS:
                            if pr == gps_lo:
                                et2 = p2.tile([P, 4, QT], BF16, tag="et2", name="et2")
                            half = 2 * (pr - gps_lo)
                            nc.scalar.activation(et2[:, half : half + 2], s_ps[:], EXP)
                            if pr == gps_lo + 1:
                                attn2 = p2.tile(
                                    [P, 4, QT], BF16, tag="attn2", name="attn2"
                                )
                                nc.gpsimd.tensor_mul(
                                    attn2[:], et2[:], eb_t[:, gps_lo : gps_lo + 2]
                                )
                                deferred = [
                                    (gps_lo + PAIRS * (j % 2) + (j // 2), attn2[:, j]) for j in range(4)
                                ]
                        else:
                            et = p2.tile([P, 2, QT], BF16, tag="et", name="et")
                            nc.scalar.activation(et[:], s_ps[:], EXP)
                            attn = p2.tile([P, 2, QT], BF16, tag="attn", name="attn")
                            nc.vector.tensor_mul(attn[:], et[:], eb_t[:, pr])
                            for j in range(2):
                                n_pv += 1
                                nc.tensor.matmul(
                                    outT_ps[:],
                                    lhsT=v_b[b][:, pr + PAIRS * j],
                                    rhs=attn[:, j],
                                    start=(n_pv == 1),
                                    stop=(n_pv == total_pv),
                                )
                    for ch, rhs_ap in deferred:
                        n_pv += 1
                        nc.tensor.matmul(
                            outT_ps[:],
                            lhsT=v_b[b][:, ch],
                            rhs=rhs_ap,
                            start=(n_pv == 1),
                            stop=(n_pv == total_pv),
                        )
                    # evacuate out^T + denominator row in one copy
                    hod = p2.tile([DH + 1, QT], BF16, tag="hod", name="hod")
                    nc.vector.tensor_copy(hod[:], outT_ps[:])
                    dT_ps = psB.tile([P, DIM], F32, tag="wo", name="dTw")
                    for s4 in range(4):
                        nc.tensor.matmul(
                            dT_ps[:, s4 : s4 + 1],
                            lhsT=hod[DH : DH + 1, s4 * P : (s4 + 1) * P],
                            rhs=ones_t[DH : DH + 1, 0:1],
                            start=True,
                            stop=True,
                        )
                    rs = p2.tile([P, 4], F32, tag="rs", name="rs")
                    with nc.allow_low_precision(reason="softmax recip"):
                        nc.vector.reciprocal(rs[:], dT_ps[:, 0:4])
                    ho = hod
                    for sq in range(4):
                        wo_ps = psB.tile([P, DIM], F32, tag="wo", name="wo")
                        nc.tensor.matmul(
                            wo_ps[:],
                            lhsT=ho[0:DH, sq * P : (sq + 1) * P],
                            rhs=wout_t[:],
                            start=True,
                            stop=True,
                        )
                        ob = p2.tile([P, DIM], BF16, tag="ob", name="ob")
                        nc.vector.tensor_scalar_mul(
                            ob[:], wo_ps[:], rs[:, sq : sq + 1]
                        )
                        row0 = jq * QT + sq * P
                        nc.sync.dma_start(out_d[b, row0 : row0 + P, :], ob[:])

                for b in range(B):
                    phase1(b)
                for jq in range(NQT):
                    eb_t = ebp.tile([P, PAIRS, 2, QT], BF16, tag="eb", name="eb")
                    nc.sync.dma_start(eb_t[:], expb_d[jq])
                    for b in range(B):
                        phase2(jq, b, eb_t)

    nc.compile()
    return nc


def _host_inputs(x, pos_bias, W_qkv, W_out):
    """Build the per-core input maps (pure data marshalling)."""
    import ml_dtypes

    bf16 = ml_dtypes.bfloat16

    xT = np.ascontiguousarray(x.transpose(0, 2, 1))          # [B, DIM, N]
    xT4 = np.ascontiguousarray(
        xT.reshape(B, DC, P, N).transpose(0, 2, 1, 3)
    ).astype(bf16)                                           # [B, P, DC, N]

    # split-d permutation: even dims then odd dims
    perm = np.concatenate([np.arange(0, DH, 2), np.arange(1, DH, 2)])
    inv_freq = (1.0 / (10000.0 ** (np.arange(0, DH, 2, dtype=np.float32) / DH)))
    pos = np.arange(N, dtype=np.float32)
    fr = inv_freq[:, None] * pos[None, :]                     # [32, N]
    cos_h = np.cos(fr)
    sin_h = np.sin(fr)
    # rows: q-even, q-odd, k-even, k-odd halves all share the per-pair angle
    cos2 = np.concatenate([cos_h] * 4, axis=0).astype(bf16)   # [128, N]
    sin2 = np.concatenate([sin_h] * 4, axis=0).astype(bf16)

    onesv = np.ones((P, NCH), dtype=np.float32).astype(bf16)

    scale = np.float32(DH ** -0.5)
    in_maps = []
    for h in range(HEADS):
        Wq = (W_qkv[:, h * DH : (h + 1) * DH] * scale)[:, perm]   # split-d
        Wk = W_qkv[:, DIM + h * DH : DIM + (h + 1) * DH][:, perm]
        Wv = W_qkv[:, 2 * DIM + h * DH : 2 * DIM + (h + 1) * DH]
        # rotate-half in split layout: s_e = -c_o, s_o = c_e
        Wq_s = np.concatenate([-Wq[:, 32:64], Wq[:, 0:32]], axis=1)
        Wk_s = np.concatenate([-Wk[:, 32:64], Wk[:, 0:32]], axis=1)
        Wc = np.concatenate([Wq, Wk], axis=1)                 # [512, 128]
        Ws = np.concatenate([Wq_s, Wk_s], axis=1)             # [512, 128]
        wqk = np.ascontiguousarray(
            np.stack(
                [
                    Wc.reshape(DC, P, P).transpose(1, 0, 2),
                    Ws.reshape(DC, P, P).transpose(1, 0, 2),
                ],
                axis=1,
            )
        ).astype(bf16)                                        # [P, 2, DC, P]
        wv = np.ascontiguousarray(
            Wv.reshape(DC, P, DH).transpose(1, 0, 2)
        ).astype(bf16)                                        # [P, DC, DH]
        wout = np.ascontiguousarray(W_out[h * DH : (h + 1) * DH, :]).astype(bf16)
        ebT = np.exp(pos_bias[h].T.astype(np.float32))        # [k, q]
        expb = np.ascontiguousarray(
            ebT.reshape(2, PAIRS, P, NQT, QT).transpose(3, 2, 1, 0, 4)
        ).astype(bf16)                                        # [NQT, P, PAIRS, 2, QT]
        in_maps.append(
            {
                "xT4": xT4,
                "wqk": wqk,
                "wv": wv,
                "wout": wout,
                "expb": expb,
                "cos2": cos2,
                "sin2": sin2,
                "onesv": onesv,
            }
        )
    return in_maps


def kernel(x, pos_bias, W_qkv, W_out, _trace=False):
    from concourse.bass_utils import run_bass_kernel_spmd

    x = np.asarray(x, dtype=np.float32)
    pos_bias = np.asarray(pos_bias, dtype=np.float32)
    W_qkv = np.asarray(W_qkv, dtype=np.float32)
    W_out = np.asarray(W_out, dtype=np.float32)

    if "nc" not in _CACHE:
        _CACHE["nc"] = _build()
    nc = _CACHE["nc"]

    in_maps = _host_inputs(x, pos_bias, W_qkv, W_out)
    try:
        res = run_bass_kernel_spmd(
            nc, in_maps, core_ids=list(range(HEADS)), trace=_trace
        )
    except ModuleNotFoundError:
        res = run_bass_kernel_spmd(
            nc, in_maps, core_ids=list(range(HEADS)), trace=False
        )
    out = np.zeros((B, N, DIM), dtype=np.float32)
    for rmap in res.results:
        out += rmap["out"].astype(np.float32)
    if _trace:
        return out, res
    return out


if __name__ == "__main__":
    rng = np.random.default_rng(0)
    x = rng.standard_normal((B, N, DIM), dtype=np.float32)
    pb = rng.standard_normal((HEADS, N, N), dtype=np.float32)
    wq = rng.standard_normal((DIM, 3 * DIM), dtype=np.float32) * DIM**-0.5
    wo = rng.standard_normal((DIM, DIM), dtype=np.float32) * DIM**-0.5
    o = kernel(x, pb, wq, wo)
    print("kernel ran, out std:", o.std())
